# revision 6
# baseline (speedup 1.0000x reference)
"""Trainium2 Bass kernel for causal multi-head attention with RoPE.

Reference computation (per nn.Module):
    q,k,v = x@Wq.T, x@Wk.T, x@Wv.T  -> (B,H,S,HD)
    q,k = rope(q), rope(k)
    out = softmax(causal(q@k.T/sqrt(HD))) @ v  -> merge heads -> @ Wo.T
    returns (out, k_rope, v)

Sharding: tensor-parallel over heads. 8 cores x 2 heads each. Each core
computes q/k/v projections for its 2 heads from the full x, runs causal
SDPA, and produces a partial o_proj output (contracted over its 128
context dims). Host sums the 8 partials; k/v cache shards concatenate.

Kernel layout choices:
  - Work in "transposed" activation layout (head_dim on partitions, seq on
    free): scores are computed directly as scores.T (k_pos on partitions,
    q_pos on free), which lets p@v run with NO transposes of p.
  - Softmax denominator: an extra ones-column appended to v (v_aug) makes
    the same matmul that accumulates ctx also produce the exp-row-sums.
    Normalization is deferred to after p@v (linearity) and applied per
    head before o_proj mixes heads.
  - RoPE in transposed layout: pair partner (d=2i <-> 2i+1) lives in the
    same 32-partition quadrant, so DVE stream_shuffle with mask [1,0,3,2..]
    provides the rotated operand; cos/sin tables are pre-expanded on host
    to (128, S) with signs folded in, and the 1/sqrt(HD) scale folded into
    the q tables.
  - All big matmuls use float32r (full PE rate at N>=256, ~1e-4 rounding).
"""

import numpy as np
from contextlib import ExitStack

B, S, D, H, HD = 4, 2048, 1024, 16, 64
NCORES = 8
HPC = H // NCORES  # heads per core = 2

DEFAULT_CFG = dict(B=B, S=S, D=D, QT=1024)


def _split_waits(nc, mybir, maxw=1):
    """walrus in this env allows only ONE sem wait per instruction; hoist
    excess waits onto NoOps inserted just before, on the same engine."""
    n_new = 0
    for f in nc.m.functions:
        for bb in f.blocks:
            insts = bb.instructions
            newlist = []
            changed = False
            for inst in insts:
                si = inst.sync_info
                if si is not None and si.on_wait is not None and len(si.on_wait) > maxw:
                    waits = list(si.on_wait)
                    extra, keep = waits[:-maxw], waits[-maxw:]
                    for j in range(0, len(extra), maxw):
                        n_new += 1
                        nop = mybir.InstNoOp(
                            name=f"I-waitsplit-{n_new}", ins=[], outs=[]
                        )
                        nop.engine = inst.engine
                        nop.sync_info = mybir.SyncInfo(
                            on_wait=extra[j : j + maxw], on_update=[]
                        )
                        newlist.append(nop)
                    si.on_wait = keep
                    changed = True
                newlist.append(inst)
            if changed:
                bb.instructions[:] = newlist
    return n_new


def build_nc(cfg=None, split_waits=True):
    import concourse.bass as bass
    import concourse.mybir as mybir
    import concourse.tile as tile

    cfg = dict(DEFAULT_CFG, **(cfg or {}))
    B_, S_, D_, QT = cfg["B"], cfg["S"], cfg["D"], cfg["QT"]
    F32 = mybir.dt.float32
    F32R = mybir.dt.float32r
    EXPF = mybir.ActivationFunctionType.Exp

    NK = D_ // 128          # proj contraction chunks
    NSEQ = S_ // 512        # proj seq chunks per batch
    NM = S_ // 128          # 128-wide seq chunks per batch
    NQT = S_ // QT          # q tiles per batch
    NHALF = QT // 512       # 512-wide halves per q tile

    nc = bass.Bass()

    xT = nc.dram_tensor("xT", [D_, B_ * S_], F32R, kind="ExternalInput")
    wqkv = nc.dram_tensor("wqkv", [D_, 3 * 128], F32R, kind="ExternalInput")
    wo2 = nc.dram_tensor("wo2", [2, 64, D_], F32R, kind="ExternalInput")
    cosP = nc.dram_tensor("cosP", [128, S_], F32, kind="ExternalInput")
    sinP = nc.dram_tensor("sinP", [128, S_], F32, kind="ExternalInput")
    cosPq = nc.dram_tensor("cosPq", [128, S_], F32, kind="ExternalInput")
    sinPq = nc.dram_tensor("sinPq", [128, S_], F32, kind="ExternalInput")
    tri_d = nc.dram_tensor("tri", [128, 128], F32R, kind="ExternalInput")
    ident_d = nc.dram_tensor("ident", [128, 128], F32, kind="ExternalInput")
    ones_d = nc.dram_tensor("ones", [128, 64], F32R, kind="ExternalInput")

    out_d = nc.dram_tensor("outp", [B_ * S_, D_], F32, kind="ExternalOutput")
    k_d = nc.dram_tensor("k_out", [B_, HPC, S_, HD], F32, kind="ExternalOutput")
    v_d = nc.dram_tensor("v_out", [B_, HPC, S_, HD], F32, kind="ExternalOutput")

    PAIRSWAP = [i ^ 1 for i in range(32)]

    with tile.TileContext(nc) as tc:
        with ExitStack() as ctx:
            const = ctx.enter_context(tc.tile_pool(name="const", bufs=1))
            sb = ctx.enter_context(tc.tile_pool(name="sb", bufs=2))
            ps = ctx.enter_context(tc.tile_pool(name="ps", bufs=2, space="PSUM"))

            # ---- constants ----
            w_sb = const.tile([128, NK, 3 * 128], F32R)
            nc.sync.dma_start(
                w_sb[:], wqkv.rearrange("(kk p) m -> p kk m", p=128)
            )
            wo_sb = []
            for hl in range(2):
                t = const.tile([64, D_], F32R, name=f"wo_sb{hl}")
                nc.sync.dma_start(t[:], wo2[hl])
                wo_sb.append(t)
            cosP_sb = const.tile([128, S_], F32)
            nc.sync.dma_start(cosP_sb[:], cosP[:])
            sinP_sb = const.tile([128, S_], F32)
            nc.sync.dma_start(sinP_sb[:], sinP[:])
            cosPq_sb = const.tile([128, S_], F32)
            nc.sync.dma_start(cosPq_sb[:], cosPq[:])
            sinPq_sb = const.tile([128, S_], F32)
            nc.sync.dma_start(sinPq_sb[:], sinPq[:])
            tri_sb = const.tile([128, 128], F32R)
            nc.sync.dma_start(tri_sb[:], tri_d[:])
            ident_sb = const.tile([128, 128], F32)
            nc.sync.dma_start(ident_sb[:], ident_d[:])
            ones_sb = const.tile([128, 64], F32R)
            nc.sync.dma_start(ones_sb[:], ones_d[:])

            for b in range(B_):
                # ======== projections + rope (transposed layout) ========
                qT = sb.tile([128, S_], F32R, tag="qT", bufs=2, name=f"qT{b}")
                kT = sb.tile([128, S_], F32R, tag="kT", bufs=2, name=f"kT{b}")
                vT = sb.tile([128, S_], F32R, tag="vT", bufs=2, name=f"vT{b}")
                for n in range(NSEQ):
                    nsl = slice(n * 512, (n + 1) * 512)
                    xts = []
                    for kk in range(NK):
                        xt = sb.tile(
                            [128, 512],
                            F32R,
                            tag="xt",
                            bufs=NK + 2,
                            name=f"xt{b}_{n}_{kk}",
                        )
                        nc.sync.dma_start(
                            xt[:],
                            xT[
                                kk * 128 : (kk + 1) * 128,
                                b * S_ + n * 512 : b * S_ + (n + 1) * 512,
                            ],
                        )
                        xts.append(xt)
                    for ti, (dst, cos_t, sin_t) in enumerate(
                        (
                            (qT, cosPq_sb, sinPq_sb),
                            (kT, cosP_sb, sinP_sb),
                            (vT, None, None),
                        )
                    ):
                        pp = ps.tile(
                            [128, 512], F32, tag="big", bufs=2, name=f"pp{b}_{n}_{ti}"
                        )
                        for kk in range(NK):
                            nc.tensor.matmul(
                                pp[:],
                                w_sb[:, kk, ti * 128 : (ti + 1) * 128],
                                xts[kk][:],
                                start=(kk == 0),
                                stop=(kk == NK - 1),
                            )
                        if cos_t is None:
                            nc.vector.tensor_copy(vT[:, nsl], pp[:])
                        else:
                            tmp = sb.tile(
                                [128, 512], F32, tag="tmpA", bufs=2, name=f"tmA{b}{n}{ti}"
                            )
                            nc.vector.tensor_tensor(
                                out=tmp[:],
                                in0=pp[:],
                                in1=cos_t[:, nsl],
                                op=mybir.AluOpType.mult,
                            )
                            sw = sb.tile(
                                [128, 512], F32, tag="tmpB", bufs=2, name=f"tmB{b}{n}{ti}"
                            )
                            nc.vector.stream_shuffle(sw[:], pp[:], PAIRSWAP)
                            nc.vector.tensor_tensor(
                                out=sw[:],
                                in0=sw[:],
                                in1=sin_t[:, nsl],
                                op=mybir.AluOpType.mult,
                            )
                            nc.vector.tensor_tensor(
                                out=dst[:, nsl],
                                in0=tmp[:],
                                in1=sw[:],
                                op=mybir.AluOpType.add,
                            )

                # ======== v transpose to natural layout (+ caches) ========
                vas = []
                for m in range(NM):
                    msl = slice(m * 128, (m + 1) * 128)
                    tp = ps.tile([128, 128], F32, tag="big", bufs=2, name=f"vtp{b}_{m}")
                    nc.tensor.transpose(tp[:], vT[:, msl].bitcast(F32), ident_sb[:])
                    va = sb.tile(
                        [128, 130], F32R, tag="vaug", bufs=NM + 4, name=f"va{b}_{m}"
                    )
                    nc.vector.tensor_copy(va[:, 0:64], tp[:, 0:64])
                    nc.vector.tensor_copy(va[:, 65:129], tp[:, 64:128])
                    nc.vector.tensor_copy(va[:, 64:65], ones_sb[:, 0:1])
                    nc.vector.tensor_copy(va[:, 129:130], ones_sb[:, 1:2])
                    vas.append(va)
                    # v cache out: (s, hl, d) iteration on both sides
                    nc.sync.dma_start(
                        v_d[b, :, msl, :].rearrange("hl s d -> s hl d"),
                        va[:, 0:130]
                        .bitcast(F32)
                        .rearrange("s (hl d) -> s hl d", hl=2, d=65)[:, :, 0:64],
                    )

                # ======== SDPA per head / q-tile ========
                for qt in range(NQT):
                    qb = qt * QT
                    nkc = (qb + QT) // 128
                    ctxns = []
                    for h in range(2):
                        hsl = slice(h * 64, (h + 1) * 64)
                        chalf = []
                        for hf in range(NHALF):
                            c = ps.tile(
                                [65, 512],
                                F32,
                                tag="ctx",
                                bufs=2 * NHALF,
                                name=f"ctx{b}_{qt}_{h}_{hf}",
                            )
                            chalf.append(c)
                        sc_tiles = {}

                        def emit_sc(kc):
                            kb = kc * 128
                            qoff = max(0, kb - qb)
                            scp = ps.tile(
                                [128, QT],
                                F32,
                                tag="big",
                                bufs=2,
                                name=f"sc{b}_{qt}_{h}_{kc}",
                            )
                            for hf in range(NHALF):
                                s0 = max(qoff, hf * 512)
                                s1 = (hf + 1) * 512
                                if s0 >= s1:
                                    continue
                                nc.tensor.matmul(
                                    scp[:, s0:s1],
                                    kT[hsl, kb : kb + 128],
                                    qT[hsl, qb + s0 : qb + s1],
                                    start=True,
                                    stop=True,
                                )
                            sc_tiles[kc] = scp

                        emit_sc(0)
                        for kc in range(nkc):
                            kb = kc * 128
                            qoff = max(0, kb - qb)
                            if kc + 1 < nkc:
                                emit_sc(kc + 1)
                            scp = sc_tiles.pop(kc)
                            ex = sb.tile(
                                [128, QT],
                                F32R,
                                tag="exp",
                                bufs=3,
                                name=f"ex{b}_{qt}_{h}_{kc}",
                            )
                            nc.scalar.activation(ex[:, qoff:QT], scp[:, qoff:QT], EXPF)
                            if kb >= qb:
                                nc.vector.tensor_tensor(
                                    out=ex[:, qoff : qoff + 128],
                                    in0=ex[:, qoff : qoff + 128],
                                    in1=tri_sb[:],
                                    op=mybir.AluOpType.mult,
                                )
                            for hf in range(NHALF):
                                s0 = max(qoff, hf * 512)
                                s1 = (hf + 1) * 512
                                if s0 >= s1:
                                    continue
                                last_kc = (qb + s1) // 128 - 1
                                nc.tensor.matmul(
                                    chalf[hf][:, s0 - hf * 512 : 512],
                                    vas[kc][:, h * 65 : h * 65 + 65],
                                    ex[:, s0:s1],
                                    start=(kc == 0),
                                    stop=(kc == last_kc),
                                )
                        # normalize: ctxn = ctx * (1/sums) broadcast over rows
                        ctxn = sb.tile(
                            [64, QT], F32R, tag="ctxn", bufs=3, name=f"cn{b}_{qt}_{h}"
                        )
                        for hf in range(NHALF):
                            rs = sb.tile(
                                [65, 512], F32R, tag="rs", bufs=2, name=f"rs{b}{qt}{h}{hf}"
                            )
                            with nc.allow_low_precision(
                                reason="f32r is 4-byte; reciprocal of softmax sums"
                            ):
                                nc.vector.reciprocal(
                                    rs[64:65, :], chalf[hf][64:65, :]
                                )
                            rr = ps.tile(
                                [64, 512], F32, tag="big", bufs=2, name=f"rr{b}{qt}{h}{hf}"
                            )
                            nc.tensor.matmul(
                                rr[:],
                                ones_sb[64:65, 0:64],
                                rs[64:65, :],
                                start=True,
                                stop=True,
                            )
                            rrs = sb.tile(
                                [64, 512],
                                F32,
                                tag="rrs",
                                bufs=2,
                                name=f"rrs{b}{qt}{h}{hf}",
                            )
                            nc.vector.tensor_copy(rrs[:], rr[:])
                            nc.vector.tensor_tensor(
                                out=ctxn[:, hf * 512 : (hf + 1) * 512],
                                in0=chalf[hf][0:64, :],
                                in1=rrs[:],
                                op=mybir.AluOpType.mult,
                            )
                        ctxns.append(ctxn)

                    # ======== o_proj partial: out += ctxn_h.T @ wo_h ========
                    for sub in range(QT // 128):
                        ssl = slice(sub * 128, (sub + 1) * 128)
                        for no in range(D_ // 512):
                            osl = slice(no * 512, (no + 1) * 512)
                            op = ps.tile(
                                [128, 512],
                                F32,
                                tag="big",
                                bufs=2,
                                name=f"op{b}_{qt}_{sub}_{no}",
                            )
                            for h in range(2):
                                nc.tensor.matmul(
                                    op[:],
                                    ctxns[h][:, ssl],
                                    wo_sb[h][:, osl],
                                    start=(h == 0),
                                    stop=(h == 1),
                                )
                            osb = sb.tile(
                                [128, 512],
                                F32,
                                tag="outsb",
                                bufs=3,
                                name=f"ob{b}_{qt}_{sub}_{no}",
                            )
                            nc.vector.tensor_copy(osb[:], op[:])
                            r0 = b * S_ + qb + sub * 128
                            nc.sync.dma_start(out_d[r0 : r0 + 128, osl], osb[:])

                # ======== k transpose for cache output ========
                for m in range(NM):
                    msl = slice(m * 128, (m + 1) * 128)
                    tp = ps.tile([128, 128], F32, tag="big", bufs=2, name=f"ktp{b}_{m}")
                    nc.tensor.transpose(tp[:], kT[:, msl].bitcast(F32), ident_sb[:])
                    kn = sb.tile([128, 128], F32, tag="knat", bufs=2, name=f"kn{b}_{m}")
                    nc.vector.tensor_copy(kn[:], tp[:])
                    nc.sync.dma_start(
                        k_d[b, :, msl, :].rearrange("hl s d -> s hl d"),
                        kn[:].rearrange("s (hl d) -> s hl d", hl=2),
                    )

    if split_waits:
        _split_waits(nc, mybir)
    return nc


def _host_inputs(x, cos, sin, wq, wk, wv, wo):
    """Build per-core input maps."""
    f32 = np.float32
    B_, S_, D_ = x.shape
    xT = np.ascontiguousarray(x.reshape(B_ * S_, D_).T).astype(f32)
    cc = np.ascontiguousarray(cos.T).astype(f32)  # (32, S)
    ss = np.ascontiguousarray(sin.T).astype(f32)
    cos64 = np.repeat(cc, 2, axis=0)  # (64, S)
    sin64 = np.empty_like(cos64)
    sin64[0::2] = -ss
    sin64[1::2] = ss
    cosP = np.tile(cos64, (2, 1))
    sinP = np.tile(sin64, (2, 1))
    scale = np.float32(HD ** -0.5)
    shared = {
        "xT": xT,
        "cosP": cosP,
        "sinP": sinP,
        "cosPq": cosP * scale,
        "sinPq": sinP * scale,
        "tri": np.triu(np.ones((128, 128), f32)),
        "ident": np.eye(128, dtype=f32),
        "ones": np.ones((128, 64), f32),
    }
    in_maps = []
    for c in range(NCORES):
        r0 = c * HPC * HD
        rows = slice(r0, r0 + HPC * HD)
        wq_l = np.ascontiguousarray(wq[rows, :].T)
        wk_l = np.ascontiguousarray(wk[rows, :].T)
        wv_l = np.ascontiguousarray(wv[rows, :].T)
        wqkv = np.concatenate([wq_l, wk_l, wv_l], axis=1).astype(f32)
        wo2 = np.stack(
            [
                np.ascontiguousarray(wo[:, r0 + h * HD : r0 + (h + 1) * HD].T)
                for h in range(HPC)
            ]
        ).astype(f32)
        in_maps.append(dict(shared, wqkv=wqkv, wo2=wo2))
    return in_maps


_NC_CACHE = {}


def kernel(x, cos, sin, wq, wk, wv, wo):
    from concourse.bass_utils import run_bass_kernel_spmd

    x = np.asarray(x, np.float32)
    cos = np.asarray(cos, np.float32)
    sin = np.asarray(sin, np.float32)
    wq = np.asarray(wq, np.float32)
    wk = np.asarray(wk, np.float32)
    wv = np.asarray(wv, np.float32)
    wo = np.asarray(wo, np.float32)

    if "nc" not in _NC_CACHE:
        _NC_CACHE["nc"] = build_nc()
    nc = _NC_CACHE["nc"]

    in_maps = _host_inputs(x, cos, sin, wq, wk, wv, wo)
    res = run_bass_kernel_spmd(nc, in_maps, list(range(NCORES)))

    out = np.zeros((B * S, D), np.float64)
    k_full = np.empty((B, H, S, HD), np.float32)
    v_full = np.empty((B, H, S, HD), np.float32)
    for c in range(NCORES):
        r = res.results[c]
        out += r["outp"].astype(np.float64)
        k_full[:, c * HPC : (c + 1) * HPC] = r["k_out"]
        v_full[:, c * HPC : (c + 1) * HPC] = r["v_out"]
    out = out.astype(np.float32).reshape(B, S, D)
    return out, k_full, v_full


# revision 23
# speedup vs baseline: 17959.7678x; 17959.7678x over previous
"""Trainium2 Bass kernel for causal multi-head attention with RoPE.

Reference computation (per nn.Module):
    q,k,v = x@Wq.T, x@Wk.T, x@Wv.T  -> (B,H,S,HD)
    q,k = rope(q), rope(k)
    out = softmax(causal(q@k.T/sqrt(HD))) @ v  -> merge heads -> @ Wo.T
    returns (out, k_rope, v)

Sharding: tensor-parallel over heads. 8 cores x 2 heads each. Each core
computes q/k/v projections for its 2 heads from the full x, runs causal
SDPA, and produces a partial o_proj output (contracted over its 128
context dims). Host sums the 8 partials; k/v cache shards concatenate.

Kernel layout choices:
  - Transposed activation layout (head_dim on partitions, seq on free):
    scores are computed directly as scores.T (k_pos on partitions, q_pos
    on free), so p@v needs NO transposes of the softmax matrix.
  - Softmax denominator: an extra ones-column in the stationary v operand
    makes the p@v matmul also produce exp-row-sums. Normalization is
    deferred past p@v (linearity) and applied per head before o_proj
    mixes heads; 1/sums = Exp(-Ln(sums)) on the scalar engine (both
    functions live in one ACT table set).
  - All matmuls are bf16 with fp32 PSUM accumulation, and every matmul
    feeds the FULL 128-row PE array (per-head q/k zero-padded to K=128,
    v_aug stationary widened to M=128, contexts of both heads stacked via
    an SBUF->SBUF DMA so o_proj contracts K=128): half-array matmuls leave
    the PE HAM clock-gate cold (1.2 GHz); full-array ones run at 2.4 GHz.
  - RoPE pairs (2i, 2i+1) sit in the same 32-partition quadrant, so DVE
    stream_shuffle mask [1,0,3,2,...] yields the rotated operand; host
    pre-expands cos/sin to (128,S) with signs folded, and folds the
    1/sqrt(HD) scale into the q tables.
"""

import numpy as np
from contextlib import ExitStack

B, S, D, H, HD = 4, 2048, 1024, 16, 64
NCORES = 8
HPC = H // NCORES  # heads per core = 2

DEFAULT_CFG = dict(B=B, S=S, D=D, QT=512)


def _split_waits(nc, mybir, maxw=1):
    """walrus in this env allows only ONE sem wait per instruction; hoist
    excess waits onto NoOps inserted just before, on the same engine."""
    n_new = 0
    for f in nc.m.functions:
        for bb in f.blocks:
            insts = bb.instructions
            newlist = []
            changed = False
            for inst in insts:
                si = inst.sync_info
                if si is not None and si.on_wait is not None and len(si.on_wait) > maxw:
                    waits = list(si.on_wait)
                    extra, keep = waits[:-maxw], waits[-maxw:]
                    for j in range(0, len(extra), maxw):
                        n_new += 1
                        nop = mybir.InstNoOp(
                            name=f"I-waitsplit-{n_new}", ins=[], outs=[]
                        )
                        nop.engine = inst.engine
                        nop.sync_info = mybir.SyncInfo(
                            on_wait=extra[j : j + maxw], on_update=[]
                        )
                        newlist.append(nop)
                    si.on_wait = keep
                    changed = True
                newlist.append(inst)
            if changed:
                bb.instructions[:] = newlist
    return n_new


def build_nc(cfg=None, split_waits=True):
    import concourse.bass as bass
    import concourse.mybir as mybir
    import concourse.tile as tile

    cfg = dict(DEFAULT_CFG, **(cfg or {}))
    B_, S_, D_, QT = cfg["B"], cfg["S"], cfg["D"], cfg["QT"]
    F32 = mybir.dt.float32
    BF16 = mybir.dt.bfloat16
    EXPF = mybir.ActivationFunctionType.Exp
    LOGF = mybir.ActivationFunctionType.Ln
    MULT = mybir.AluOpType.mult
    ADD = mybir.AluOpType.add

    NK = D_ // 128          # proj contraction chunks
    NSEQ = S_ // 512        # proj seq chunks per batch
    NM = S_ // 128          # 128-wide seq chunks per batch
    NQT = S_ // QT          # q tiles per batch
    NHALF = QT // 512       # 512-wide halves per q tile

    nc = bass.Bass()

    xT = nc.dram_tensor("xT", [D_, B_ * S_], BF16, kind="ExternalInput")
    wqkv = nc.dram_tensor("wqkv", [D_, 3 * 128], BF16, kind="ExternalInput")
    wor = nc.dram_tensor("wor", [128, D_], BF16, kind="ExternalInput")
    cosP = nc.dram_tensor("cosP", [128, S_], F32, kind="ExternalInput")
    sinP = nc.dram_tensor("sinP", [128, S_], F32, kind="ExternalInput")
    cosPq = nc.dram_tensor("cosPq", [128, S_], F32, kind="ExternalInput")
    sinPq = nc.dram_tensor("sinPq", [128, S_], F32, kind="ExternalInput")
    tri_d = nc.dram_tensor("tri", [128, 128], BF16, kind="ExternalInput")
    ident_d = nc.dram_tensor("ident", [128, 128], BF16, kind="ExternalInput")
    ones_d = nc.dram_tensor("ones", [128, 64], BF16, kind="ExternalInput")

    out_d = nc.dram_tensor("outp", [B_ * S_, D_], F32, kind="ExternalOutput")
    k_d = nc.dram_tensor("k_out", [B_, HPC, S_, HD], F32, kind="ExternalOutput")
    v_d = nc.dram_tensor("v_out", [B_, HPC, S_, HD], F32, kind="ExternalOutput")

    PAIRSWAP = [i ^ 1 for i in range(32)]

    with tile.TileContext(nc) as tc:
        with ExitStack() as ctx:
            const = ctx.enter_context(tc.tile_pool(name="const", bufs=1))
            sb = ctx.enter_context(tc.tile_pool(name="sb", bufs=2))
            ps = ctx.enter_context(tc.tile_pool(name="ps", bufs=2, space="PSUM"))

            # ---- constants ----
            w_sb = const.tile([128, NK, 3 * 128], BF16)
            nc.sync.dma_start(w_sb[:], wqkv.rearrange("(kk p) m -> p kk m", p=128))
            wo_sb = const.tile([128, D_], BF16)
            nc.sync.dma_start(wo_sb[:], wor[:])
            cosP_sb = const.tile([128, S_], F32)
            nc.sync.dma_start(cosP_sb[:], cosP[:])
            sinP_sb = const.tile([128, S_], F32)
            nc.sync.dma_start(sinP_sb[:], sinP[:])
            cosPq_sb = const.tile([128, S_], F32)
            nc.sync.dma_start(cosPq_sb[:], cosPq[:])
            sinPq_sb = const.tile([128, S_], F32)
            nc.sync.dma_start(sinPq_sb[:], sinPq[:])
            tri_sb = const.tile([128, 128], BF16)
            nc.sync.dma_start(tri_sb[:], tri_d[:])
            ident_sb = const.tile([128, 128], BF16)
            nc.sync.dma_start(ident_sb[:], ident_d[:])
            ones_sb = const.tile([128, 64], BF16)
            nc.sync.dma_start(ones_sb[:], ones_d[:])

            for b in range(B_):
                # ======== projections + rope (transposed layout) ========
                qT = sb.tile([128, S_], BF16, tag="qT", bufs=2, name=f"qT{b}")
                kT = sb.tile([128, S_], BF16, tag="kT", bufs=2, name=f"kT{b}")
                vT = sb.tile([128, S_], BF16, tag="vT", bufs=2, name=f"vT{b}")
                qz, kz = [], []
                for hh in range(2):
                    qzt = sb.tile([128, S_], BF16, tag=f"qz{hh}", bufs=2,
                                  name=f"qz{hh}_{b}")
                    kzt = sb.tile([128, S_], BF16, tag=f"kz{hh}", bufs=2,
                                  name=f"kz{hh}_{b}")
                    nc.gpsimd.memset(qzt[64:128, :], 0.0)
                    nc.gpsimd.memset(kzt[64:128, :], 0.0)
                    qz.append(qzt)
                    kz.append(kzt)
                for n in range(NSEQ):
                    nsl = slice(n * 512, (n + 1) * 512)
                    xts = []
                    for kk in range(NK):
                        xt = sb.tile(
                            [128, 512], BF16, tag="xt", bufs=NK + 2,
                            name=f"xt{b}_{n}_{kk}",
                        )
                        nc.sync.dma_start(
                            xt[:],
                            xT[kk * 128 : (kk + 1) * 128,
                               b * S_ + n * 512 : b * S_ + (n + 1) * 512],
                        )
                        xts.append(xt)
                    for ti, (dst, cos_t, sin_t) in enumerate(
                        (
                            (qT, cosPq_sb, sinPq_sb),
                            (kT, cosP_sb, sinP_sb),
                            (vT, None, None),
                        )
                    ):
                        pp = ps.tile(
                            [128, 512], F32, tag="big", bufs=5, name=f"pp{b}_{n}_{ti}"
                        )
                        for kk in range(NK):
                            nc.tensor.matmul(
                                pp[:],
                                w_sb[:, kk, ti * 128 : (ti + 1) * 128],
                                xts[kk][:],
                                start=(kk == 0),
                                stop=(kk == NK - 1),
                            )
                        if cos_t is None:
                            nc.vector.tensor_copy(vT[:, nsl], pp[:])
                        else:
                            tmp = sb.tile(
                                [128, 512], F32, tag="tmpA", bufs=2,
                                name=f"tmA{b}{n}{ti}",
                            )
                            nc.vector.tensor_tensor(
                                out=tmp[:], in0=pp[:], in1=cos_t[:, nsl], op=MULT
                            )
                            sw = sb.tile(
                                [128, 512], F32, tag="tmpB", bufs=2,
                                name=f"tmB{b}{n}{ti}",
                            )
                            nc.vector.stream_shuffle(sw[:], pp[:], PAIRSWAP)
                            nc.vector.tensor_tensor(
                                out=sw[:], in0=sw[:], in1=sin_t[:, nsl], op=MULT
                            )
                            nc.vector.tensor_tensor(
                                out=dst[:, nsl], in0=tmp[:], in1=sw[:], op=ADD
                            )
                            # incrementally build per-head zero-padded copies
                            # (full K=128 keeps the PE HAM clock-gate warm)
                            z0, z1 = (qz[0], qz[1]) if ti == 0 else (kz[0], kz[1])
                            nc.vector.tensor_copy(z0[0:64, nsl], dst[0:64, nsl])
                            nc.sync.dma_start(z1[0:64, nsl], dst[64:128, nsl])

                # ======== v transpose to natural layout (+ caches) ========
                vas = []
                for m in range(NM):
                    msl = slice(m * 128, (m + 1) * 128)
                    tp = ps.tile([128, 128], BF16, tag="big", bufs=5,
                                 name=f"vtp{b}_{m}")
                    nc.tensor.transpose(tp[:], vT[:, msl], ident_sb[:])
                    # [v_h0(64) | 1 | v_h1(64) | 1 | pad(63)]; lhsT slices are
                    # [0:128] (h0) and [65:193] (h1) -> M=128 (full array)
                    va = sb.tile([128, 194], BF16, tag="vaug", bufs=NM + 4,
                                 name=f"va{b}_{m}")
                    nc.vector.tensor_copy(va[:, 0:64], tp[:, 0:64])
                    nc.vector.tensor_copy(va[:, 65:129], tp[:, 64:128])
                    nc.vector.tensor_copy(va[:, 64:65], ones_sb[:, 0:1])
                    nc.vector.tensor_copy(va[:, 129:130], ones_sb[:, 1:2])
                    nc.gpsimd.memset(va[:, 130:194], 0.0)
                    vas.append(va)
                    # v cache out needs f32: separate copy from the psum
                    vn = sb.tile([128, 128], F32, tag="knat", bufs=4,
                                 name=f"vn{b}_{m}")
                    nc.scalar.copy(vn[:], tp[:])
                    nc.sync.dma_start(
                        v_d[b, :, msl, :].rearrange("hl s d -> s hl d"),
                        vn[:].rearrange("s (hl d) -> s hl d", hl=2),
                    )

                # ======== SDPA per q-tile, heads interleaved ========
                oproj_pending = []

                def emit_oproj(cstack_, qb_):
                    for sub in range(QT // 128):
                        ssl = slice(sub * 128, (sub + 1) * 128)
                        for no in range(D_ // 512):
                            osl = slice(no * 512, (no + 1) * 512)
                            op = ps.tile([128, 512], F32, tag="big", bufs=5,
                                         name=f"op{b}_{qb_}_{sub}_{no}")
                            nc.tensor.matmul(
                                op[:], cstack_[:, ssl], wo_sb[:, osl],
                                start=True, stop=True,
                            )
                            osb = sb.tile([128, 512], F32, tag="outsb", bufs=3,
                                          name=f"ob{b}_{qb_}_{sub}_{no}")
                            nc.vector.tensor_copy(osb[:], op[:])
                            r0 = b * S_ + qb_ + sub * 128
                            nc.sync.dma_start(out_d[r0 : r0 + 128, osl], osb[:])

                for qt in range(NQT):
                    qb = qt * QT
                    nkc = (qb + QT) // 128
                    cstack = sb.tile([128, QT], BF16, tag="cstack", bufs=2,
                                     name=f"cs{b}_{qt}")
                    ctx2 = [
                        ps.tile([128, 512], F32, tag="ctx", bufs=3,
                                name=f"ctx{b}_{qt}_{h}")
                        for h in range(2)
                    ]
                    lhs_vs = ((0, 128), (65, 193))
                    sc_tiles = {}

                    def emit_sc(kc, h):
                        kb = kc * 128
                        qoff = max(0, kb - qb)
                        scp = ps.tile([128, QT], F32, tag="big", bufs=5,
                                      name=f"sc{b}_{qt}_{h}_{kc}")
                        nc.tensor.matmul(
                            scp[:, qoff:QT],
                            kz[h][:, kb : kb + 128],
                            qz[h][:, qb + qoff : qb + QT],
                            start=True,
                            stop=True,
                        )
                        sc_tiles[(kc, h)] = scp

                    emit_sc(0, 0)
                    emit_sc(0, 1)
                    for kc in range(nkc):
                        kb = kc * 128
                        qoff = max(0, kb - qb)
                        for h in range(2):
                            if h == 0 and kc + 1 < nkc:
                                emit_sc(kc + 1, 0)
                                emit_sc(kc + 1, 1)
                            scp = sc_tiles.pop((kc, h))
                            ex = sb.tile([128, QT], BF16, tag="exp", bufs=4,
                                         name=f"ex{b}_{qt}_{h}_{kc}")
                            nc.scalar.activation(ex[:, qoff:QT], scp[:, qoff:QT], EXPF)
                            if kb >= qb:
                                nc.vector.tensor_tensor(
                                    out=ex[:, qoff : qoff + 128],
                                    in0=ex[:, qoff : qoff + 128],
                                    in1=tri_sb[:],
                                    op=MULT,
                                )
                            nc.tensor.matmul(
                                ctx2[h][:, qoff:QT],
                                vas[kc][:, lhs_vs[h][0] : lhs_vs[h][1]],
                                ex[:, qoff:QT],
                                start=(kc == 0),
                                stop=(kc == nkc - 1),
                            )
                    # normalization: 1/sums = Exp(-Ln(sums)) on ACT, then
                    # scale ctx rows and write into the head-stacked tile
                    ctxn1 = sb.tile([64, QT], BF16, tag="ctxn", bufs=2,
                                    name=f"cn{b}_{qt}")
                    for h in range(2):
                        ls = sb.tile([65, 512], F32, tag="ls", bufs=2,
                                     name=f"ls{b}{qt}{h}")
                        nc.scalar.activation(ls[64:65, :], ctx2[h][64:65, :], LOGF)
                        rs = sb.tile([65, 512], BF16, tag="rs", bufs=2,
                                     name=f"rs{b}{qt}{h}")
                        nc.scalar.activation(rs[64:65, :], ls[64:65, :], EXPF,
                                             scale=-1.0)
                        rr = ps.tile([64, 512], F32, tag="big", bufs=5,
                                     name=f"rr{b}{qt}{h}")
                        nc.tensor.matmul(rr[:], ones_sb[64:65, 0:64], rs[64:65, :],
                                         start=True, stop=True)
                        rrs = sb.tile([64, 512], F32, tag="rrs", bufs=2,
                                      name=f"rrs{b}{qt}{h}")
                        nc.vector.tensor_copy(rrs[:], rr[:])
                        dst = cstack[0:64, :] if h == 0 else ctxn1[:, :]
                        nc.vector.tensor_tensor(
                            out=dst, in0=ctx2[h][0:64, :], in1=rrs[:], op=MULT
                        )
                    # cross-partition stack of head 1's context
                    nc.sync.dma_start(cstack[64:128, :], ctxn1[:, :])

                    # o_proj deferred one q-tile so the next tile's score
                    # matmuls aren't queued behind the norm-chain latency
                    oproj_pending.append((cstack, qb))
                    if len(oproj_pending) > 1:
                        emit_oproj(*oproj_pending.pop(0))

                # ======== k transpose for cache output ========
                for m in range(NM):
                    msl = slice(m * 128, (m + 1) * 128)
                    tp = ps.tile([128, 128], BF16, tag="big", bufs=5,
                                 name=f"ktp{b}_{m}")
                    nc.tensor.transpose(tp[:], kT[:, msl], ident_sb[:])
                    kn = sb.tile([128, 128], F32, tag="knat", bufs=4,
                                 name=f"kn{b}_{m}")
                    nc.scalar.copy(kn[:], tp[:])
                    nc.sync.dma_start(
                        k_d[b, :, msl, :].rearrange("hl s d -> s hl d"),
                        kn[:].rearrange("s (hl d) -> s hl d", hl=2),
                    )

                while oproj_pending:
                    emit_oproj(*oproj_pending.pop(0))

    if split_waits:
        _split_waits(nc, mybir)
    return nc


def _host_inputs(x, cos, sin, wq, wk, wv, wo):
    """Build per-core input maps."""
    import ml_dtypes

    f32 = np.float32
    bf16 = ml_dtypes.bfloat16
    B_, S_, D_ = x.shape
    xT = np.ascontiguousarray(x.reshape(B_ * S_, D_).T).astype(bf16)
    cc = np.ascontiguousarray(cos.T).astype(f32)  # (32, S)
    ss = np.ascontiguousarray(sin.T).astype(f32)
    cos64 = np.repeat(cc, 2, axis=0)  # (64, S)
    sin64 = np.empty_like(cos64)
    sin64[0::2] = -ss
    sin64[1::2] = ss
    cosP = np.tile(cos64, (2, 1))
    sinP = np.tile(sin64, (2, 1))
    scale = np.float32(HD ** -0.5)
    shared = {
        "xT": xT,
        "cosP": cosP,
        "sinP": sinP,
        "cosPq": cosP * scale,
        "sinPq": sinP * scale,
        "tri": np.triu(np.ones((128, 128), np.float32)).astype(bf16),
        "ident": np.eye(128, dtype=f32).astype(bf16),
        "ones": np.ones((128, 64), f32).astype(bf16),
    }
    in_maps = []
    for c in range(NCORES):
        r0 = c * HPC * HD
        rows = slice(r0, r0 + HPC * HD)
        wq_l = np.ascontiguousarray(wq[rows, :].T)
        wk_l = np.ascontiguousarray(wk[rows, :].T)
        wv_l = np.ascontiguousarray(wv[rows, :].T)
        wqkv = np.concatenate([wq_l, wk_l, wv_l], axis=1).astype(bf16)
        wor = np.ascontiguousarray(wo[:, rows].T).astype(bf16)  # (128, D)
        in_maps.append(dict(shared, wqkv=wqkv, wor=wor))
    return in_maps


_NC_CACHE = {}


def kernel(x, cos, sin, wq, wk, wv, wo):
    from concourse.bass_utils import run_bass_kernel_spmd

    x = np.asarray(x, np.float32)
    cos = np.asarray(cos, np.float32)
    sin = np.asarray(sin, np.float32)
    wq = np.asarray(wq, np.float32)
    wk = np.asarray(wk, np.float32)
    wv = np.asarray(wv, np.float32)
    wo = np.asarray(wo, np.float32)

    if "nc" not in _NC_CACHE:
        _NC_CACHE["nc"] = build_nc()
    nc = _NC_CACHE["nc"]

    in_maps = _host_inputs(x, cos, sin, wq, wk, wv, wo)
    res = run_bass_kernel_spmd(nc, in_maps, list(range(NCORES)))

    out = np.zeros((B * S, D), np.float64)
    k_full = np.empty((B, H, S, HD), np.float32)
    v_full = np.empty((B, H, S, HD), np.float32)
    for c in range(NCORES):
        r = res.results[c]
        out += r["outp"].astype(np.float64)
        k_full[:, c * HPC : (c + 1) * HPC] = r["k_out"]
        v_full[:, c * HPC : (c + 1) * HPC] = r["v_out"]
    out = out.astype(np.float32).reshape(B, S, D)
    return out, k_full, v_full
